# revision 31
# baseline (speedup 1.0000x reference)
"""Trainium2 Bass kernel for nn_CPA_43 (dense transformer block, CPA attention).

Data-parallel over batch: B=256 sharded as 32 samples per core across 8 cores.
All weights replicated. Two on-chip stages per core (split by activation-table
set: exp for the softmaxes, gelu for the MLP; one reload each per pass):
  stage 1: LN1/LN2, Q/K/V projections, channel-softmax(q), position-softmax(k),
           context/attention matmuls, Wr + residual -> f3out (spilled to DRAM
           in bf16), LN3 statistics on the fly (bn_stats).
  stage 2: LN3 apply, MLP (W1 -> gelu -> W2) + residual -> output.

All six large matmul families (Q, K, V, R, W1, W2) run in fp8e4m3 with
DoubleRow perf mode (2 fp8 contraction rows per PE cell). Weights are
host-scaled by 64 so sigma~0.02 entries clear the fp8 subnormal threshold;
the 1/64 de-scale is folded into the downstream activation's scale or the
residual-add's scalar. The attention inner path stays bf16. Overall rel err
~1.24e-2 vs the 2e-2 gate (fp8 MLP dominates the error).

Layout choices avoid all transposes of q and k:
 - Q is projected channel-major for a PAIR of samples at a time
   (out = Wq.T @ x3_cm, free dim 512); the channel-softmax sum is an 8-wide
   PE matmul against a head-selector matrix, the 1/sum is partition-broadcast
   back with a second tiny matmul, and q_exp is normalized on DVE before the
   attention matmul.
 - K is projected token-major like V; the position-softmax sum is a PE matmul
   against a block-ones selector, and 1/sum is transposed (tiny) to
   channel-major per-partition scalars folded into the context scaling.

Engine assignment (GPSIMD avoided entirely: it shares the DVE SBUF port and
runs ~2x slower per element, and BIR forbids it PSUM access): Act does exps,
PSUM->SBUF copies, gelu, and LN applies via per-partition scale/bias Identity
activations; DVE does stats, reciprocals, q-normalize, and residual adds.

Bias-row preload matmuls (bv, br, b2) are emitted only when those rows are
nonzero on the host (they are all zero for this problem's reference), saving
~144 PE matmuls per pass.
"""

import numpy as np

B, N3, N4, DIM, HEADS, MLP_DIM = 256, 256, 64, 512, 8, 2048
N_CORES = 8
BSH = B // N_CORES  # samples per core
EPS = 1e-5
SG4 = 8  # f4-group size (samples per K/V block)
WS = 64.0  # fp8 weight scale

_BUILD_CACHE = {}


def _host_prep(inputs):
    """Fold LN gains + positional projections into weights/biases (exact)."""
    import ml_dtypes

    f = {k: np.asarray(v, dtype=np.float64) for k, v in inputs.items()}
    pos3 = f["pos3"][0]  # [N3, DIM]
    pos4 = f["pos4"][0]  # [N4, DIM]

    bf16 = ml_dtypes.bfloat16
    fp8 = ml_dtypes.float8_e4m3

    def q8(x):
        return np.ascontiguousarray(
            np.clip(x * WS, -240.0, 240.0).astype(fp8))

    wq = q8(f["ln1_g"][:, None] * f["Wq"])
    wk = q8(f["ln2_g"][:, None] * f["Wk"])
    wv = q8(f["ln2_g"][:, None] * f["Wv"])
    wr = q8(f["Wr"])
    w1 = q8(f["ln3_g"][:, None] * f["W1"])
    w2 = q8(f["W2"])

    biasq = ((f["ln1_b"][None, :] + pos3) @ f["Wq"] + f["bq"]) * WS  # [N3, DIM]
    biask = ((f["ln2_b"][None, :] + pos4) @ f["Wk"] + f["bk"]) * WS  # [N4, DIM]
    biasv = (f["ln2_b"] @ f["Wv"] + f["bv"]) * WS  # [DIM]
    bias1 = (f["ln3_b"] @ f["W1"] + f["b1"]).astype(np.float32)  # [MLP] unscaled
    br = f["br"] * WS
    b2 = f["b2"] * WS

    # bias1 as [128, 16]: column hc holds biases for hidden channels hc*128..+128
    bias1_cm = np.ascontiguousarray(bias1.reshape(MLP_DIM // 128, 128).T.astype(np.float32))

    # selector matrices for softmax reductions / broadcasts
    p = np.arange(128)
    hsel = np.zeros((128, 4, 8), dtype=np.float64)  # channel-sum: head of q-chunk
    hexp = np.zeros((8, 4, 128), dtype=np.float64)  # head -> partition broadcast
    for cc in range(4):
        hsel[p, cc, 2 * cc + p // 64] = 1.0
        hexp[2 * cc + p // 64, cc, p] = 1.0
    i2 = np.zeros((64, 128), dtype=np.float64)  # bias row-block doubling
    i2[p[:128] % 64, p[:128]] = 1.0
    blk8 = np.zeros((128, 4, 8), dtype=np.float64)  # position-sum selector
    for t in range(4):
        blk8[p, t, 2 * t + p // 64] = 1.0

    return {
        "wq": wq, "wk": wk, "wv": wv, "wr": wr, "w1": w1, "w2": w2,
        "biasq_cm": np.ascontiguousarray(np.tile(biasq.T, (1, 2)).astype(bf16)),  # [DIM, 2*N3]
        "biask_tm": np.ascontiguousarray(biask.astype(bf16)),    # [N4, DIM]
        "biasv_row": np.ascontiguousarray(biasv[None, :].astype(np.float32)),
        "br_row": np.ascontiguousarray(br[None, :].astype(np.float32)),
        "b2_row": np.ascontiguousarray(b2[None, :].astype(np.float32)),
        "bias1_cm": bias1_cm,
        "ones_col": np.ones((1, 128), dtype=np.float32),
        "identbf": np.ascontiguousarray(np.eye(128).astype(bf16)),
        "hsel": np.ascontiguousarray(hsel.astype(bf16)),
        "hexp": np.ascontiguousarray(hexp.astype(bf16)),
        "i2": np.ascontiguousarray(i2.astype(bf16)),
        "blk8": np.ascontiguousarray(blk8.astype(bf16)),
    }


def _build(n_samples, repeat=1, zero_rows=False):
    """Build the Bacc module for one core processing `n_samples` samples.

    `repeat` re-runs the whole computation that many times back-to-back —
    used only for wall-clock timing amplification in test.py."""
    import concourse.bacc as bacc
    import concourse.tile as tile
    import concourse.mybir as mybir
    from concourse.bass import AP  # noqa: F401

    # Restrict activation-table-set choices (see module docstring).
    if not hasattr(bacc, "_orig_get_activation_tables"):
        bacc._orig_get_activation_tables = bacc.get_activation_tables

        def _gat(arch):
            full = bacc._orig_get_activation_tables(arch)
            keep = {"natural_log_exp_and_others", "gelu_and_others"}
            return {n: (s if n in keep else set()) for n, s in full.items()}

        bacc.get_activation_tables = _gat

    F32 = mybir.dt.float32
    F32R = mybir.dt.float32r
    BF16 = mybir.dt.bfloat16
    FP8 = mybir.dt.float8e4
    DR = mybir.MatmulPerfMode.DoubleRow
    ALU = mybir.AluOpType
    ACTF = mybir.ActivationFunctionType
    RS = 1.0 / WS

    NS = n_samples
    assert NS % SG4 == 0
    NG4 = NS // SG4      # f4 groups
    NG2 = NS // 2        # mlp groups of 2 samples

    nc = bacc.Bacc("TRN2", debug=False, num_devices=N_CORES)

    f3 = nc.dram_tensor("f3", [NS, N3, DIM], F32, kind="ExternalInput").ap()
    f4 = nc.dram_tensor("f4", [NS, N4, DIM], F32, kind="ExternalInput").ap()
    wq = nc.dram_tensor("wq", [DIM, DIM], FP8, kind="ExternalInput").ap()
    wk = nc.dram_tensor("wk", [DIM, DIM], FP8, kind="ExternalInput").ap()
    wv = nc.dram_tensor("wv", [DIM, DIM], FP8, kind="ExternalInput").ap()
    wr = nc.dram_tensor("wr", [DIM, DIM], FP8, kind="ExternalInput").ap()
    w1 = nc.dram_tensor("w1", [DIM, MLP_DIM], FP8, kind="ExternalInput").ap()
    w2 = nc.dram_tensor("w2", [MLP_DIM, DIM], FP8, kind="ExternalInput").ap()
    biasq_cm = nc.dram_tensor("biasq_cm", [DIM, 2 * N3], BF16, kind="ExternalInput").ap()
    biask_tm = nc.dram_tensor("biask_tm", [N4, DIM], BF16, kind="ExternalInput").ap()
    biasv_row = nc.dram_tensor("biasv_row", [1, DIM], F32R, kind="ExternalInput").ap()
    br_row = nc.dram_tensor("br_row", [1, DIM], F32R, kind="ExternalInput").ap()
    b2_row = nc.dram_tensor("b2_row", [1, DIM], F32R, kind="ExternalInput").ap()
    bias1_cm = nc.dram_tensor("bias1_cm", [128, MLP_DIM // 128], F32, kind="ExternalInput").ap()
    ones_col = nc.dram_tensor("ones_col", [1, 128], F32R, kind="ExternalInput").ap()
    identbf = nc.dram_tensor("identbf", [128, 128], BF16, kind="ExternalInput").ap()
    hsel = nc.dram_tensor("hsel", [128, 4, 8], BF16, kind="ExternalInput").ap()
    hexp = nc.dram_tensor("hexp", [8, 4, 128], BF16, kind="ExternalInput").ap()
    i2 = nc.dram_tensor("i2", [64, 128], BF16, kind="ExternalInput").ap()
    blk8 = nc.dram_tensor("blk8", [128, 4, 8], BF16, kind="ExternalInput").ap()
    out = nc.dram_tensor("out", [NS, N3, DIM], F32, kind="ExternalOutput").ap()

    with tile.TileContext(nc) as tc:
        # ---- pools alive for the whole kernel ----
        with (
            tc.tile_pool(name="consts", bufs=1) as cpool,
            tc.tile_pool(name="wattn", bufs=1) as wpool,
            tc.tile_pool(name="stats", bufs=1) as spool,
            tc.tile_pool(name="dram", bufs=1, space="DRAM") as dpool,
        ):
            identb_sb = cpool.tile([128, 128], BF16, tag="identb")
            nc.sync.dma_start(identb_sb[:], identbf)
            hsel_sb = cpool.tile([128, 4, 8], BF16, tag="hsel")
            nc.sync.dma_start(hsel_sb[:], hsel)
            hexp_sb = cpool.tile([8, 4, 128], BF16, tag="hexp")
            nc.sync.dma_start(hexp_sb[:], hexp)
            i2_sb = cpool.tile([64, 128], BF16, tag="i2")
            nc.sync.dma_start(i2_sb[:], i2)
            blk8_sb = cpool.tile([128, 4, 8], BF16, tag="blk8")
            nc.sync.dma_start(blk8_sb[:], blk8)
            eps_sb = cpool.tile([128, 1], F32, tag="eps")
            nc.vector.memset(eps_sb[:], EPS)
            ones_sb = cpool.tile([1, 128], F32R, tag="ones")
            nc.sync.dma_start(ones_sb[:], ones_col)
            bvrow_sb = cpool.tile([1, DIM], F32R, tag="bvrow")
            nc.sync.dma_start(bvrow_sb[:], biasv_row)
            brrow_sb = cpool.tile([1, DIM], F32R, tag="brrow")
            b2row_sb = cpool.tile([1, DIM], F32R, tag="b2row")
            bqc_sb = cpool.tile([128, 4, 2 * N3], BF16, tag="bqc")
            bkt_sb = cpool.tile([64, DIM], BF16, tag="bkt")
            nc.sync.dma_start(bkt_sb[:], biask_tm)
            b1_sb = cpool.tile([128, MLP_DIM // 128], F32, tag="b1")
            # persistent block-diagonal context tiles (off-diagonal stays zero)
            ctxbd2 = cpool.tile([128, 2, 4, 128], BF16, tag="ctxbd")
            nc.vector.memset(ctxbd2[:], 0.0)

            wq_sb = wpool.tile([128, 4, DIM], FP8, tag="wq")
            wk_sb = wpool.tile([128, 4, DIM], FP8, tag="wk")
            wv_sb = wpool.tile([128, 4, DIM], FP8, tag="wv")
            wr_sb = wpool.tile([128, 4, DIM], FP8, tag="wr")
            w1_sb = wpool.tile([128, 4, MLP_DIM], FP8, tag="w1")
            w2_sb = wpool.tile([128, 8, 2, DIM], FP8, tag="w2")

            # LN3 (mean, var) per token-chunk column, filled during stage 1
            stats3 = spool.tile([128, 2 * NS, 2], F32, tag="stats3")

            f3o_dram = dpool.tile([NS, N3, DIM], BF16, tag="f3spill")

            for _rep in range(repeat):
                # ================= STAGE 1 =================
                with (
                    tc.tile_pool(name="s1_sb", bufs=2) as p1,
                    tc.tile_pool(name="s1_sb3", bufs=3) as p13,
                    tc.tile_pool(name="kv", bufs=3) as pkv,
                    tc.tile_pool(name="ps_tp", bufs=1, space="PSUM") as ps_tp,
                    tc.tile_pool(name="ps_mmq", bufs=2, space="PSUM") as ps_mmq,
                    tc.tile_pool(name="ps_mmr", bufs=2, space="PSUM") as ps_mmr,
                    tc.tile_pool(name="ps_sm", bufs=1, space="PSUM") as ps_sm,
                    tc.tile_pool(name="ps_ksum", bufs=1, space="PSUM") as ps_ksum,
                    tc.tile_pool(name="ps_ctx", bufs=1, space="PSUM") as ps_ctx,
                ):
                    def inv_std_from_var(var_view, sinv_view, n_cols, tag):
                        """sinv = exp(-0.5*ln(var+eps)) on [128, n_cols] views."""
                        lnv = p1.tile([128, n_cols], F32, tag=f"lnv_{tag}")
                        nc.scalar.activation(lnv[:], var_view, ACTF.Ln, bias=eps_sb[:])
                        nc.scalar.activation(sinv_view, lnv[:], ACTF.Exp, scale=-0.5)

                    def f4_block(g):
                            # ---------- f4 block: SG4 samples ----------
                            ntb = SG4 // 2  # token-chunks of 128 (2 samples each)
                            x4 = p1.tile([128, ntb, DIM], F32, tag="x4")
                            for t in range(ntb):
                                nc.sync.dma_start(
                                    x4[:, t, :],
                                    f4[SG4 * g + 2 * t: SG4 * g + 2 * t + 2].rearrange(
                                        "a b d -> (a b) d"
                                    ),
                                )
                            mv4 = p1.tile([128, ntb, 2], F32, tag="mv4")
                            for t in range(ntb):
                                bns = p1.tile([128, 6], F32, tag="bns4")
                                nc.vector.bn_stats(bns[:], x4[:, t, :])
                                nc.vector.bn_aggr(mv4[:, t, :], bns[:])
                            negm4 = p1.tile([128, ntb], F32, tag="negm4")
                            nc.vector.tensor_scalar_mul(negm4[:], mv4[:, :, 0], -1.0)
                            sinv4 = p1.tile([128, ntb], F32, tag="sinv4")
                            inv_std_from_var(mv4[:, :, 1], sinv4[:], ntb, "s4")
                            negmn4 = p1.tile([128, ntb], F32, tag="negmn4")
                            nc.vector.tensor_tensor(negmn4[:], negm4[:], sinv4[:], op=ALU.mult)
                            x4h = p1.tile([128, ntb, DIM], BF16, tag="x4h")
                            for t in range(ntb):
                                nc.scalar.activation(
                                    x4h[:, t, :], x4[:, t, :], ACTF.Identity,
                                    bias=negmn4[:, t: t + 1], scale=sinv4[:, t: t + 1],
                                )
                            # transpose to channel-major [512ch, SG4*64 tok]
                            x4c = p1.tile([128, 4, SG4 * N4], FP8, tag="x4c", bufs=3)
                            for cc in range(4):
                                ptpf = ps_tp.tile([128, 512], F32R, tag="tp", name="ptpf")
                                ptp = ptpf.bitcast(BF16)
                                for t in range(ntb):
                                    nc.tensor.transpose(
                                        ptp[:, t * 128:(t + 1) * 128],
                                        x4h[:, t, cc * 128:(cc + 1) * 128],
                                        identb_sb[:],
                                    )
                                nc.scalar.activation(x4c[:, cc, :], ptp[:, 0:512], ACTF.Copy)
                            # K projection (token-major out) + bias preload + exp
                            k_tm = pkv.tile([128, ntb, DIM], BF16, tag="k_tm")
                            ksum = ps_ksum.tile([8, DIM], F32, tag="ksum")
                            for t in range(ntb):
                                pkt = ps_mmr.tile([128, 512], F32, tag="mmr")
                                nc.tensor.matmul(pkt[:], i2_sb[:], bkt_sb[:], start=True, stop=False)
                                for j in range(2):
                                    nc.tensor.matmul(
                                        pkt[:],
                                        x4c[:, 2 * j: 2 * j + 2, t * 128:(t + 1) * 128],
                                        wk_sb[:, 2 * j: 2 * j + 2, :],
                                        start=False, stop=(j == 1), perf_mode=DR,
                                    )
                                nc.scalar.activation(k_tm[:, t, :], pkt[:], ACTF.Exp, scale=RS)
                                nc.tensor.matmul(
                                    ksum[:], blk8_sb[:, t, :], k_tm[:, t, :],
                                    start=(t == 0), stop=(t == ntb - 1),
                                    skip_group_check=True,
                                )
                            kre = pkv.tile([8, DIM], BF16, tag="kre")
                            with nc.allow_low_precision(reason="softmax 1/sum in bf16"):
                                nc.vector.reciprocal(kre[:], ksum[:])
                            # transpose 1/ksum to channel-major per-partition scalars
                            ckp = ps_ctx.tile([128, 144], F32R, tag="ctxpkr")
                            pkr = ckp.bitcast(BF16)
                            for cc in range(4):
                                nc.tensor.transpose(
                                    pkr[:, 256 + cc * 8:256 + (cc + 1) * 8],
                                    kre[:, cc * 128:(cc + 1) * 128],
                                    identb_sb[0:8, 0:8],
                                )
                            kr_cm = pkv.tile([128, 4, 8], F32, tag="kr_cm")
                            nc.vector.tensor_copy(kr_cm[:], pkr[:, 256:288].rearrange("p (c s) -> p c s", c=4))
                            # V projection (token-major out) + bias preload
                            v_tm = pkv.tile([128, ntb, DIM], BF16, tag="v_tm")
                            for t in range(ntb):
                                pv = ps_mmr.tile([128, 512], F32, tag="mmr")
                                if not zero_rows:
                                    nc.tensor.matmul(pv[:], ones_sb[:], bvrow_sb[:], start=True, stop=False)
                                for j in range(2):
                                    nc.tensor.matmul(
                                        pv[:],
                                        x4c[:, 2 * j: 2 * j + 2, t * 128:(t + 1) * 128],
                                        wv_sb[:, 2 * j: 2 * j + 2, :],
                                        start=(zero_rows and j == 0), stop=(j == 1), perf_mode=DR,
                                    )
                                nc.scalar.activation(v_tm[:, t, :], pv[:], ACTF.Copy, scale=RS)

                            return k_tm, v_tm, kr_cm

                    def x3_load(s):
                        x3 = p13.tile([128, 2, DIM], F32, tag="x3", name="x3", bufs=6)
                        for t in range(2):
                            nc.sync.dma_start(
                                x3[:, t, :], f3[s, t * 128:(t + 1) * 128, :]
                            )
                        return x3

                    x3state = {}
                    if _rep == 0:
                        nc.sync.dma_start(wk_sb[:], wk.rearrange("(c p) d -> p c d", p=128))
                        nc.sync.dma_start(wv_sb[:], wv.rearrange("(c p) d -> p c d", p=128))
                    for _s in range(min(4, NS)):
                        x3state[_s] = x3_load(_s)
                    kvstate = {}
                    kvstate[0] = f4_block(0)
                    # deferred loads: first needed ~8-12us in (Q/Wr of sample 0)
                    nc.sync.dma_start(bqc_sb[:], biasq_cm.rearrange("(c p) t -> p c t", p=128))
                    nc.sync.dma_start(wq_sb[:], wq.rearrange("(c p) d -> p c d", p=128))
                    nc.sync.dma_start(wr_sb[:], wr.rearrange("(c p) d -> p c d", p=128))
                    nc.sync.dma_start(brrow_sb[:], br_row)
                    if NG4 > 1:
                        kvstate[1] = f4_block(1)
                    # W1/W2 loads deferred here so startup DMA bandwidth goes
                    # to the first groups' activations and attention weights.
                    nc.sync.dma_start(w1_sb[:], w1.rearrange("(c p) d -> p c d", p=128))
                    nc.sync.dma_start(w2_sb[:], w2.rearrange("(c i p) d -> p c i d", p=128, i=2))
                    nc.sync.dma_start(b1_sb[:], bias1_cm)
                    nc.sync.dma_start(b2row_sb[:], b2_row)
                    for g in range(NG4):
                        k_tm, v_tm, kr_cm = kvstate.pop(g)
                        # ---------- f3 blocks: SG4 samples ----------
                        for sp in range(SG4 // 2):
                            if sp == 2 and g + 2 < NG4:
                                kvstate[g + 2] = f4_block(g + 2)
                            s0 = SG4 * g + 2 * sp
                            xs, x3hs = [], []
                            for sl in range(2):
                                s = s0 + sl
                                x3 = x3state.pop(s)
                                if s + 4 < NS:
                                    x3state[s + 4] = x3_load(s + 4)
                                mv1 = p1.tile([128, 2, 2], F32, tag="mv1", bufs=3)
                                for t in range(2):
                                    bns1 = p1.tile([128, 6], F32, tag="bns1", bufs=3)
                                    nc.vector.bn_stats(bns1[:], x3[:, t, :])
                                    nc.vector.bn_aggr(mv1[:, t, :], bns1[:])
                                negm1 = p1.tile([128, 2], F32, tag="negm1", bufs=3)
                                nc.vector.tensor_scalar_mul(negm1[:], mv1[:, :, 0], -1.0)
                                sinv1 = p1.tile([128, 2], F32, tag="sinv1", bufs=3)
                                inv_std_from_var(mv1[:, :, 1], sinv1[:], 2, "s1")
                                negmn1 = p1.tile([128, 2], F32, tag="negmn1", bufs=3)
                                nc.vector.tensor_tensor(negmn1[:], negm1[:], sinv1[:], op=ALU.mult)
                                x3h = p1.tile([128, 2, DIM], BF16, tag="x3h", bufs=4)
                                for t in range(2):
                                    nc.scalar.activation(
                                        x3h[:, t, :], x3[:, t, :], ACTF.Identity,
                                        bias=negmn1[:, t: t + 1], scale=sinv1[:, t: t + 1],
                                    )
                                xs.append(x3); x3hs.append(x3h)
                            # pair channel-major LN'd x3: [512ch, 512tok]
                            x3c = p1.tile([128, 4, 512], FP8, tag="x3c", bufs=3)
                            for cc in range(4):
                                ptpf3 = ps_tp.tile([128, 512], F32R, tag="tp", name="ptpf3")
                                ptp = ptpf3.bitcast(BF16)
                                for sl in range(2):
                                    for t in range(2):
                                        nc.tensor.transpose(
                                            ptp[:, sl * 256 + t * 128: sl * 256 + (t + 1) * 128],
                                            x3hs[sl][:, t, cc * 128:(cc + 1) * 128],
                                            identb_sb[:],
                                        )
                                nc.scalar.activation(x3c[:, cc, :], ptp[:, 0:512], ACTF.Copy)
                            # Q projection channel-major (both samples) + bias + exp
                            qexp = p1.tile([128, 4, 512], BF16, tag="qexp", bufs=3)
                            qsum = ps_sm.tile([8, DIM], F32, tag="sm")
                            for cc in range(4):
                                pqc = ps_mmq.tile([128, 512], F32, tag="mmq", name="pqc")
                                nc.tensor.matmul(
                                    pqc[:], identb_sb[:], bqc_sb[:, cc, :],
                                    start=True, stop=False,
                                )
                                for j in range(2):
                                    nc.tensor.matmul(
                                        pqc[:],
                                        wq_sb[:, 2 * j: 2 * j + 2, cc * 128:(cc + 1) * 128],
                                        x3c[:, 2 * j: 2 * j + 2, :],
                                        start=False, stop=(j == 1), perf_mode=DR,
                                    )
                                nc.scalar.activation(qexp[:, cc, :], pqc[:], ACTF.Exp, scale=RS)
                                nc.tensor.matmul(
                                    qsum[:], hsel_sb[:, cc, :], qexp[:, cc, :],
                                    start=(cc == 0), stop=(cc == 3),
                                    skip_group_check=True,
                                )
                            qre = p1.tile([8, DIM], BF16, tag="qre", bufs=3)
                            with nc.allow_low_precision(reason="softmax 1/sum in bf16"):
                                nc.vector.reciprocal(qre[:], qsum[:])
                            # attention per head-pair, both samples
                            att2 = p1.tile([128, 4, 512], FP8, tag="att_cm", bufs=3)
                            for hp in range(4):
                                arb = ps_mmq.tile([128, 512], F32, tag="mmq", name="arb")
                                nc.tensor.matmul(
                                    arb[:], hexp_sb[:, hp, :], qre[:],
                                    start=True, stop=True,
                                )
                                apt = ps_mmq.tile([128, 512], F32, tag="mmq", name="apt")
                                qn2 = p1.tile([128, 512], BF16, tag="qn", bufs=4)
                                nc.vector.tensor_tensor(
                                    qn2[:], qexp[:, hp, :], arb[:], op=ALU.mult,
                                )
                                for sl in range(2):
                                    s = s0 + sl
                                    si = 2 * sp + sl
                                    tb = si // 2
                                    pb = (si % 2) * 64
                                    ck2 = ps_ctx.tile([128, 144], F32R, tag="ctxpkr")
                                    pctx = ck2.bitcast(F32)[:, 0:128]
                                    nc.tensor.matmul(
                                        pctx,
                                        k_tm[pb:pb + 64, tb, hp * 128:(hp + 1) * 128],
                                        v_tm[pb:pb + 64, tb, hp * 128:(hp + 1) * 128],
                                        start=True, stop=True,
                                    )
                                    ctxbd = ctxbd2[:, sl]
                                    for hh in range(2):
                                        nc.vector.tensor_scalar_mul(
                                            ctxbd[hh * 64:(hh + 1) * 64, hp, hh * 64:(hh + 1) * 64],
                                            pctx[hh * 64:(hh + 1) * 64, hh * 64:(hh + 1) * 64],
                                            kr_cm[hh * 64:(hh + 1) * 64, hp, si: si + 1],
                                        )
                                    nc.tensor.matmul(
                                        apt[:, sl * 256:(sl + 1) * 256],
                                        ctxbd[:, hp, :], qn2[:, sl * 256:(sl + 1) * 256],
                                        start=True, stop=True,
                                    )
                                nc.scalar.activation(att2[:, hp, :], apt[:], ACTF.Copy)
                            # Wr + residual -> f3out (+ LN3 stats via bn_stats)
                            for sl in range(2):
                                s = s0 + sl
                                for t in range(2):
                                    po = ps_mmr.tile([128, 512], F32, tag="mmr")
                                    if not zero_rows:
                                        nc.tensor.matmul(po[:], ones_sb[:], brrow_sb[:], start=True, stop=False)
                                    for j in range(2):
                                        nc.tensor.matmul(
                                            po[:],
                                            att2[:, 2 * j: 2 * j + 2, sl * 256 + t * 128: sl * 256 + (t + 1) * 128],
                                            wr_sb[:, 2 * j: 2 * j + 2, :],
                                            start=(zero_rows and j == 0), stop=(j == 1), perf_mode=DR,
                                        )
                                    f3o = p13.tile([128, DIM], BF16, tag="f3o", bufs=4)
                                    nc.vector.scalar_tensor_tensor(
                                        f3o[:], po[:], RS, xs[sl][:, t, :],
                                        op0=ALU.mult, op1=ALU.add,
                                    )
                                    bns3 = p1.tile([128, 6], F32, tag="bns3", bufs=3)
                                    nc.vector.bn_stats(bns3[:], f3o[:])
                                    nc.vector.bn_aggr(stats3[:, 2 * s + t, :], bns3[:])
                                    nc.sync.dma_start(
                                        f3o_dram[s, t * 128:(t + 1) * 128, :], f3o[:]
                                    )

                # ================= STAGE 2 =================
                with (
                    tc.tile_pool(name="s2_sb", bufs=3) as p2,
                    tc.tile_pool(name="s2_sb3", bufs=3) as p23,
                    tc.tile_pool(name="ps2_tp", bufs=1, space="PSUM") as ps2_tp,
                    tc.tile_pool(name="ps2_w1", bufs=3, space="PSUM") as ps2_w1,
                    tc.tile_pool(name="ps2_w2", bufs=4, space="PSUM") as ps2_w2,
                ):
                    # LN3 stats math for all samples at once
                    negm3 = p2.tile([128, 2 * NS], F32, tag="negm3")
                    nc.vector.tensor_scalar_mul(negm3[:], stats3[:, :, 0], -1.0)
                    lnv3 = p2.tile([128, 2 * NS], F32, tag="lnv3")
                    nc.scalar.activation(lnv3[:], stats3[:, :, 1], ACTF.Ln, bias=eps_sb[:])
                    s3 = p2.tile([128, 2 * NS], F32, tag="s3")
                    nc.scalar.activation(s3[:], lnv3[:], ACTF.Exp, scale=-0.5)

                    for g in range(NG2):
                        f3o2 = p23.tile([128, 4, DIM], BF16, tag="f3o2")
                        for c in range(4):
                            nc.sync.dma_start(
                                f3o2[:, c, :],
                                f3o_dram[2 * g + c // 2, (c % 2) * 128:(c % 2) * 128 + 128, :],
                            )
                        xoh = p2.tile([128, 4, DIM], BF16, tag="xoh")
                        for c in range(4):
                            col = 4 * g + c
                            nc.vector.tensor_scalar(
                                xoh[:, c, :], f3o2[:, c, :],
                                negm3[:, col: col + 1], s3[:, col: col + 1],
                                op0=ALU.add, op1=ALU.mult,
                            )
                        xoc = p2.tile([128, 4, DIM], FP8, tag="xoc")
                        for cc in range(4):
                            ptpf2 = ps2_tp.tile([128, 512], F32R, tag="tp2", name="ptpf2")
                            ptp = ptpf2.bitcast(BF16)
                            for c in range(4):
                                nc.tensor.transpose(
                                    ptp[:, c * 128:(c + 1) * 128],
                                    xoh[:, c, cc * 128:(cc + 1) * 128],
                                    identb_sb[:],
                                )
                            nc.vector.tensor_copy(xoc[:, cc, :], ptp[:, 0:512])
                        pf = []
                        for c in range(4):
                            pfc = ps2_w2.tile([128, 512], F32, tag="w2acc")
                            if not zero_rows:
                                nc.tensor.matmul(pfc[:], ones_sb[:], b2row_sb[:], start=True, stop=False)
                            pf.append(pfc)
                        for hcp in range(8):
                            gt2 = p23.tile([128, 2, DIM], FP8, tag="gt2")
                            for i in range(2):
                                hc = 2 * hcp + i
                                pw1 = ps2_w1.tile([128, 512], F32, tag="w1ps")
                                for j in range(2):
                                    nc.tensor.matmul(
                                        pw1[:],
                                        w1_sb[:, 2 * j: 2 * j + 2, hc * 128:(hc + 1) * 128],
                                        xoc[:, 2 * j: 2 * j + 2, :],
                                        start=(j == 0), stop=(j == 1), perf_mode=DR,
                                    )
                                nc.scalar.activation(
                                    gt2[:, i, :], pw1[:], ACTF.Gelu,
                                    bias=b1_sb[:, hc: hc + 1], scale=RS,
                                )
                            for c in range(4):
                                nc.tensor.matmul(
                                    pf[c][:],
                                    gt2[:, :, c * 128:(c + 1) * 128],
                                    w2_sb[:, hcp, :, :],
                                    start=(zero_rows and hcp == 0), stop=(hcp == 7), perf_mode=DR,
                                    skip_group_check=True,
                                )
                        for c in range(4):
                            outt = p2.tile([128, DIM], F32, tag="outt")
                            nc.vector.scalar_tensor_tensor(
                                outt[:], pf[c][:], RS, f3o2[:, c, :],
                                op0=ALU.mult, op1=ALU.add,
                            )
                            nc.sync.dma_start(
                                out[2 * g + c // 2, (c % 2) * 128:(c % 2) * 128 + 128, :],
                                outt[:],
                            )

    nc.compile()
    return nc


def _get_module(n_samples, zero_rows=False):
    key = (n_samples, zero_rows)
    if key not in _BUILD_CACHE:
        _BUILD_CACHE[key] = _build(n_samples, zero_rows=zero_rows)
    return _BUILD_CACHE[key]


def kernel(**inputs) -> np.ndarray:
    from concourse.bass_utils import run_bass_kernel_spmd

    consts = _host_prep(inputs)
    f3 = np.ascontiguousarray(np.asarray(inputs["f3"], dtype=np.float32))
    f4 = np.ascontiguousarray(np.asarray(inputs["f4"], dtype=np.float32))

    zero_rows = (
        not np.any(consts["biasv_row"]) and not np.any(consts["br_row"])
        and not np.any(consts["b2_row"])
    )
    nc = _get_module(BSH, zero_rows)
    in_maps = []
    for c in range(N_CORES):
        m = dict(consts)
        m["f3"] = np.ascontiguousarray(f3[c * BSH:(c + 1) * BSH])
        m["f4"] = np.ascontiguousarray(f4[c * BSH:(c + 1) * BSH])
        in_maps.append(m)
    res = run_bass_kernel_spmd(nc, in_maps, core_ids=list(range(N_CORES)))
    return np.concatenate([res.results[c]["out"] for c in range(N_CORES)], axis=0)


# revision 32
# speedup vs baseline: 1.0109x; 1.0109x over previous
"""Trainium2 Bass kernel for nn_CPA_43 (dense transformer block, CPA attention).

Data-parallel over batch: B=256 sharded as 32 samples per core across 8 cores.
All weights replicated. Two on-chip stages per core (split by activation-table
set: exp for the softmaxes, gelu for the MLP; one reload each per pass):
  stage 1: LN1/LN2, Q/K/V projections, channel-softmax(q), position-softmax(k),
           context/attention matmuls, Wr + residual -> f3out (spilled to DRAM
           in bf16), LN3 statistics on the fly (bn_stats).
  stage 2: LN3 apply, MLP (W1 -> gelu -> W2) + residual -> output.

All six large matmul families (Q, K, V, R, W1, W2) run in fp8e4m3 with
DoubleRow perf mode (2 fp8 contraction rows per PE cell). Weights are
host-scaled by 64 so sigma~0.02 entries clear the fp8 subnormal threshold;
the 1/64 de-scale is folded into the downstream activation's scale or the
residual-add's scalar. The attention inner path stays bf16. Overall rel err
~1.24e-2 vs the 2e-2 gate (fp8 MLP dominates the error).

Layout choices avoid all transposes of q and k:
 - Q is projected channel-major for a PAIR of samples at a time
   (out = Wq.T @ x3_cm, free dim 512); the channel-softmax sum is an 8-wide
   PE matmul against a head-selector matrix, the 1/sum is partition-broadcast
   back with a second tiny matmul, and q_exp is normalized on DVE before the
   attention matmul.
 - K is projected token-major like V; the position-softmax sum is a PE matmul
   against a block-ones selector, and 1/sum is transposed (tiny) to
   channel-major per-partition scalars folded into the context scaling.

Engine assignment (GPSIMD avoided entirely: it shares the DVE SBUF port and
runs ~2x slower per element, and BIR forbids it PSUM access): Act does exps,
PSUM->SBUF copies, gelu, and LN applies via per-partition scale/bias Identity
activations; DVE does stats, reciprocals, q-normalize, and residual adds.

Bias-row preload matmuls (bv, br, b2) are emitted only when those rows are
nonzero on the host (they are all zero for this problem's reference), saving
~144 PE matmuls per pass.
"""

import numpy as np

B, N3, N4, DIM, HEADS, MLP_DIM = 256, 256, 64, 512, 8, 2048
N_CORES = 8
BSH = B // N_CORES  # samples per core
EPS = 1e-5
SG4 = 8  # f4-group size (samples per K/V block)
WS = 64.0  # fp8 weight scale

_BUILD_CACHE = {}


def _host_prep(inputs):
    """Fold LN gains + positional projections into weights/biases (exact)."""
    import ml_dtypes

    f = {k: np.asarray(v, dtype=np.float64) for k, v in inputs.items()}
    pos3 = f["pos3"][0]  # [N3, DIM]
    pos4 = f["pos4"][0]  # [N4, DIM]

    bf16 = ml_dtypes.bfloat16
    fp8 = ml_dtypes.float8_e4m3

    def q8(x):
        return np.ascontiguousarray(
            np.clip(x * WS, -240.0, 240.0).astype(fp8))

    wq = q8(f["ln1_g"][:, None] * f["Wq"])
    wk = q8(f["ln2_g"][:, None] * f["Wk"])
    wv = q8(f["ln2_g"][:, None] * f["Wv"])
    wr = q8(f["Wr"])
    w1 = q8(f["ln3_g"][:, None] * f["W1"])
    w2 = q8(f["W2"])

    biasq = ((f["ln1_b"][None, :] + pos3) @ f["Wq"] + f["bq"]) * WS  # [N3, DIM]
    biask = ((f["ln2_b"][None, :] + pos4) @ f["Wk"] + f["bk"]) * WS  # [N4, DIM]
    biasv = (f["ln2_b"] @ f["Wv"] + f["bv"]) * WS  # [DIM]
    bias1 = (f["ln3_b"] @ f["W1"] + f["b1"]).astype(np.float32)  # [MLP] unscaled
    br = f["br"] * WS
    b2 = f["b2"] * WS

    # bias1 as [128, 16]: column hc holds biases for hidden channels hc*128..+128
    bias1_cm = np.ascontiguousarray(bias1.reshape(MLP_DIM // 128, 128).T.astype(np.float32))

    # selector matrices for softmax reductions / broadcasts
    p = np.arange(128)
    hsel = np.zeros((128, 4, 8), dtype=np.float64)  # channel-sum: head of q-chunk
    hexp = np.zeros((8, 4, 128), dtype=np.float64)  # head -> partition broadcast
    for cc in range(4):
        hsel[p, cc, 2 * cc + p // 64] = 1.0
        hexp[2 * cc + p // 64, cc, p] = 1.0
    i2 = np.zeros((64, 128), dtype=np.float64)  # bias row-block doubling
    i2[p[:128] % 64, p[:128]] = 1.0
    blk8 = np.zeros((128, 4, 8), dtype=np.float64)  # position-sum selector
    for t in range(4):
        blk8[p, t, 2 * t + p // 64] = 1.0

    return {
        "wq": wq, "wk": wk, "wv": wv, "wr": wr, "w1": w1, "w2": w2,
        "biasq_cm": np.ascontiguousarray(np.tile(biasq.T, (1, 2)).astype(bf16)),  # [DIM, 2*N3]
        "biask_tm": np.ascontiguousarray(biask.astype(bf16)),    # [N4, DIM]
        "biasv_row": np.ascontiguousarray(biasv[None, :].astype(np.float32)),
        "br_row": np.ascontiguousarray(br[None, :].astype(np.float32)),
        "b2_row": np.ascontiguousarray(b2[None, :].astype(np.float32)),
        "bias1_cm": bias1_cm,
        "ones_col": np.ones((1, 128), dtype=np.float32),
        "identbf": np.ascontiguousarray(np.eye(128).astype(bf16)),
        "hsel": np.ascontiguousarray(hsel.astype(bf16)),
        "hexp": np.ascontiguousarray(hexp.astype(bf16)),
        "i2": np.ascontiguousarray(i2.astype(bf16)),
        "blk8": np.ascontiguousarray(blk8.astype(bf16)),
    }


def _build(n_samples, repeat=1, zero_rows=False):
    """Build the Bacc module for one core processing `n_samples` samples.

    `repeat` re-runs the whole computation that many times back-to-back —
    used only for wall-clock timing amplification in test.py."""
    import concourse.bacc as bacc
    import concourse.tile as tile
    import concourse.mybir as mybir
    from concourse.bass import AP  # noqa: F401

    # Restrict activation-table-set choices (see module docstring).
    if not hasattr(bacc, "_orig_get_activation_tables"):
        bacc._orig_get_activation_tables = bacc.get_activation_tables

        def _gat(arch):
            full = bacc._orig_get_activation_tables(arch)
            keep = {"natural_log_exp_and_others", "gelu_and_others"}
            return {n: (s if n in keep else set()) for n, s in full.items()}

        bacc.get_activation_tables = _gat

    F32 = mybir.dt.float32
    F32R = mybir.dt.float32r
    BF16 = mybir.dt.bfloat16
    FP8 = mybir.dt.float8e4
    DR = mybir.MatmulPerfMode.DoubleRow
    ALU = mybir.AluOpType
    ACTF = mybir.ActivationFunctionType
    RS = 1.0 / WS

    NS = n_samples
    assert NS % SG4 == 0
    NG4 = NS // SG4      # f4 groups
    NG2 = NS // 2        # mlp groups of 2 samples

    nc = bacc.Bacc("TRN2", debug=False, num_devices=N_CORES)

    f3 = nc.dram_tensor("f3", [NS, N3, DIM], F32, kind="ExternalInput").ap()
    f4 = nc.dram_tensor("f4", [NS, N4, DIM], F32, kind="ExternalInput").ap()
    wq = nc.dram_tensor("wq", [DIM, DIM], FP8, kind="ExternalInput").ap()
    wk = nc.dram_tensor("wk", [DIM, DIM], FP8, kind="ExternalInput").ap()
    wv = nc.dram_tensor("wv", [DIM, DIM], FP8, kind="ExternalInput").ap()
    wr = nc.dram_tensor("wr", [DIM, DIM], FP8, kind="ExternalInput").ap()
    w1 = nc.dram_tensor("w1", [DIM, MLP_DIM], FP8, kind="ExternalInput").ap()
    w2 = nc.dram_tensor("w2", [MLP_DIM, DIM], FP8, kind="ExternalInput").ap()
    biasq_cm = nc.dram_tensor("biasq_cm", [DIM, 2 * N3], BF16, kind="ExternalInput").ap()
    biask_tm = nc.dram_tensor("biask_tm", [N4, DIM], BF16, kind="ExternalInput").ap()
    biasv_row = nc.dram_tensor("biasv_row", [1, DIM], F32R, kind="ExternalInput").ap()
    br_row = nc.dram_tensor("br_row", [1, DIM], F32R, kind="ExternalInput").ap()
    b2_row = nc.dram_tensor("b2_row", [1, DIM], F32R, kind="ExternalInput").ap()
    bias1_cm = nc.dram_tensor("bias1_cm", [128, MLP_DIM // 128], F32, kind="ExternalInput").ap()
    ones_col = nc.dram_tensor("ones_col", [1, 128], F32R, kind="ExternalInput").ap()
    identbf = nc.dram_tensor("identbf", [128, 128], BF16, kind="ExternalInput").ap()
    hsel = nc.dram_tensor("hsel", [128, 4, 8], BF16, kind="ExternalInput").ap()
    hexp = nc.dram_tensor("hexp", [8, 4, 128], BF16, kind="ExternalInput").ap()
    i2 = nc.dram_tensor("i2", [64, 128], BF16, kind="ExternalInput").ap()
    blk8 = nc.dram_tensor("blk8", [128, 4, 8], BF16, kind="ExternalInput").ap()
    out = nc.dram_tensor("out", [NS, N3, DIM], F32, kind="ExternalOutput").ap()

    with tile.TileContext(nc) as tc:
        # ---- pools alive for the whole kernel ----
        with (
            tc.tile_pool(name="consts", bufs=1) as cpool,
            tc.tile_pool(name="wattn", bufs=1) as wpool,
            tc.tile_pool(name="stats", bufs=1) as spool,
            tc.tile_pool(name="dram", bufs=1, space="DRAM") as dpool,
        ):
            identb_sb = cpool.tile([128, 128], BF16, tag="identb")
            nc.sync.dma_start(identb_sb[:], identbf)
            hsel_sb = cpool.tile([128, 4, 8], BF16, tag="hsel")
            nc.sync.dma_start(hsel_sb[:], hsel)
            hexp_sb = cpool.tile([8, 4, 128], BF16, tag="hexp")
            nc.sync.dma_start(hexp_sb[:], hexp)
            i2_sb = cpool.tile([64, 128], BF16, tag="i2")
            nc.sync.dma_start(i2_sb[:], i2)
            blk8_sb = cpool.tile([128, 4, 8], BF16, tag="blk8")
            nc.sync.dma_start(blk8_sb[:], blk8)
            eps_sb = cpool.tile([128, 1], F32, tag="eps")
            nc.vector.memset(eps_sb[:], EPS)
            ones_sb = cpool.tile([1, 128], F32R, tag="ones")
            nc.sync.dma_start(ones_sb[:], ones_col)
            bvrow_sb = cpool.tile([1, DIM], F32R, tag="bvrow")
            nc.sync.dma_start(bvrow_sb[:], biasv_row)
            brrow_sb = cpool.tile([1, DIM], F32R, tag="brrow")
            b2row_sb = cpool.tile([1, DIM], F32R, tag="b2row")
            bqc_sb = cpool.tile([128, 4, 2 * N3], BF16, tag="bqc")
            bkt_sb = cpool.tile([64, DIM], BF16, tag="bkt")
            nc.sync.dma_start(bkt_sb[:], biask_tm)
            b1_sb = cpool.tile([128, MLP_DIM // 128], F32, tag="b1")
            # persistent block-diagonal context tiles (off-diagonal stays zero)
            ctxbd2 = cpool.tile([128, 2, 4, 128], BF16, tag="ctxbd")
            nc.vector.memset(ctxbd2[:], 0.0)

            wq_sb = wpool.tile([128, 4, DIM], FP8, tag="wq")
            wk_sb = wpool.tile([128, 4, DIM], FP8, tag="wk")
            wv_sb = wpool.tile([128, 4, DIM], FP8, tag="wv")
            wr_sb = wpool.tile([128, 4, DIM], FP8, tag="wr")
            w1_sb = wpool.tile([128, 4, MLP_DIM], FP8, tag="w1")
            w2_sb = wpool.tile([128, 8, 2, DIM], FP8, tag="w2")

            # LN3 (mean, var) per token-chunk column, filled during stage 1
            stats3 = spool.tile([128, 2 * NS, 2], F32, tag="stats3")

            f3o_dram = dpool.tile([NS, N3, DIM], BF16, tag="f3spill")

            for _rep in range(repeat):
                # ================= STAGE 1 =================
                with (
                    tc.tile_pool(name="s1_sb", bufs=2) as p1,
                    tc.tile_pool(name="s1_sb3", bufs=3) as p13,
                    tc.tile_pool(name="kv", bufs=3) as pkv,
                    tc.tile_pool(name="ps_tp", bufs=1, space="PSUM") as ps_tp,
                    tc.tile_pool(name="ps_mmq", bufs=2, space="PSUM") as ps_mmq,
                    tc.tile_pool(name="ps_mmr", bufs=2, space="PSUM") as ps_mmr,
                    tc.tile_pool(name="ps_sm", bufs=1, space="PSUM") as ps_sm,
                    tc.tile_pool(name="ps_ksum", bufs=1, space="PSUM") as ps_ksum,
                    tc.tile_pool(name="ps_ctx", bufs=1, space="PSUM") as ps_ctx,
                ):
                    def inv_std_from_var(var_view, sinv_view, n_cols, tag):
                        """sinv = exp(-0.5*ln(var+eps)) on [128, n_cols] views."""
                        lnv = p1.tile([128, n_cols], F32, tag=f"lnv_{tag}")
                        nc.scalar.activation(lnv[:], var_view, ACTF.Ln, bias=eps_sb[:])
                        nc.scalar.activation(sinv_view, lnv[:], ACTF.Exp, scale=-0.5)

                    def f4_block(g):
                            # ---------- f4 block: SG4 samples ----------
                            ntb = SG4 // 2  # token-chunks of 128 (2 samples each)
                            x4 = p1.tile([128, ntb, DIM], F32, tag="x4")
                            for t in range(ntb):
                                nc.sync.dma_start(
                                    x4[:, t, :],
                                    f4[SG4 * g + 2 * t: SG4 * g + 2 * t + 2].rearrange(
                                        "a b d -> (a b) d"
                                    ),
                                )
                            mv4 = p1.tile([128, ntb, 2], F32, tag="mv4")
                            for t in range(ntb):
                                bns = p1.tile([128, 6], F32, tag="bns4")
                                nc.vector.bn_stats(bns[:], x4[:, t, :])
                                nc.vector.bn_aggr(mv4[:, t, :], bns[:])
                            negm4 = p1.tile([128, ntb], F32, tag="negm4")
                            nc.vector.tensor_scalar_mul(negm4[:], mv4[:, :, 0], -1.0)
                            sinv4 = p1.tile([128, ntb], F32, tag="sinv4")
                            inv_std_from_var(mv4[:, :, 1], sinv4[:], ntb, "s4")
                            negmn4 = p1.tile([128, ntb], F32, tag="negmn4")
                            nc.vector.tensor_tensor(negmn4[:], negm4[:], sinv4[:], op=ALU.mult)
                            x4h = p1.tile([128, ntb, DIM], BF16, tag="x4h")
                            for t in range(ntb):
                                nc.scalar.activation(
                                    x4h[:, t, :], x4[:, t, :], ACTF.Identity,
                                    bias=negmn4[:, t: t + 1], scale=sinv4[:, t: t + 1],
                                )
                            # transpose to channel-major [512ch, SG4*64 tok]
                            x4c = p1.tile([128, 4, SG4 * N4], FP8, tag="x4c", bufs=3)
                            for cc in range(4):
                                ptpf = ps_tp.tile([128, 512], F32R, tag="tp", name="ptpf")
                                ptp = ptpf.bitcast(BF16)
                                for t in range(ntb):
                                    nc.tensor.transpose(
                                        ptp[:, t * 128:(t + 1) * 128],
                                        x4h[:, t, cc * 128:(cc + 1) * 128],
                                        identb_sb[:],
                                    )
                                nc.scalar.activation(x4c[:, cc, :], ptp[:, 0:512], ACTF.Copy)
                            # K projection (token-major out) + bias preload + exp
                            k_tm = pkv.tile([128, ntb, DIM], BF16, tag="k_tm")
                            ksum = ps_ksum.tile([8, DIM], F32, tag="ksum")
                            for t in range(ntb):
                                pkt = ps_mmr.tile([128, 512], F32, tag="mmr")
                                nc.tensor.matmul(pkt[:], i2_sb[:], bkt_sb[:], start=True, stop=False)
                                for j in range(2):
                                    nc.tensor.matmul(
                                        pkt[:],
                                        x4c[:, 2 * j: 2 * j + 2, t * 128:(t + 1) * 128],
                                        wk_sb[:, 2 * j: 2 * j + 2, :],
                                        start=False, stop=(j == 1), perf_mode=DR,
                                    )
                                nc.scalar.activation(k_tm[:, t, :], pkt[:], ACTF.Exp, scale=RS)
                                nc.tensor.matmul(
                                    ksum[:], blk8_sb[:, t, :], k_tm[:, t, :],
                                    start=(t == 0), stop=(t == ntb - 1),
                                    skip_group_check=True,
                                )
                            kre = pkv.tile([8, DIM], BF16, tag="kre")
                            with nc.allow_low_precision(reason="softmax 1/sum in bf16"):
                                nc.vector.reciprocal(kre[:], ksum[:])
                            # transpose 1/ksum to channel-major per-partition scalars
                            ckp = ps_ctx.tile([128, 144], F32R, tag="ctxpkr")
                            pkr = ckp.bitcast(BF16)
                            for cc in range(4):
                                nc.tensor.transpose(
                                    pkr[:, 256 + cc * 8:256 + (cc + 1) * 8],
                                    kre[:, cc * 128:(cc + 1) * 128],
                                    identb_sb[0:8, 0:8],
                                )
                            kr_cm = pkv.tile([128, 4, 8], F32, tag="kr_cm")
                            nc.vector.tensor_copy(kr_cm[:], pkr[:, 256:288].rearrange("p (c s) -> p c s", c=4))
                            # V projection (token-major out) + bias preload
                            v_tm = pkv.tile([128, ntb, DIM], BF16, tag="v_tm")
                            for t in range(ntb):
                                pv = ps_mmr.tile([128, 512], F32, tag="mmr")
                                if not zero_rows:
                                    nc.tensor.matmul(pv[:], ones_sb[:], bvrow_sb[:], start=True, stop=False)
                                for j in range(2):
                                    nc.tensor.matmul(
                                        pv[:],
                                        x4c[:, 2 * j: 2 * j + 2, t * 128:(t + 1) * 128],
                                        wv_sb[:, 2 * j: 2 * j + 2, :],
                                        start=(zero_rows and j == 0), stop=(j == 1), perf_mode=DR,
                                    )
                                nc.scalar.activation(v_tm[:, t, :], pv[:], ACTF.Copy, scale=RS)

                            return k_tm, v_tm, kr_cm

                    def x3_load(s):
                        x3 = p13.tile([128, 2, DIM], F32, tag="x3", name="x3", bufs=4)
                        for t in range(2):
                            nc.sync.dma_start(
                                x3[:, t, :], f3[s, t * 128:(t + 1) * 128, :]
                            )
                        return x3

                    x3state = {}
                    if _rep == 0:
                        nc.sync.dma_start(wk_sb[:], wk.rearrange("(c p) d -> p c d", p=128))
                        nc.sync.dma_start(wv_sb[:], wv.rearrange("(c p) d -> p c d", p=128))
                    x3state[0] = x3_load(0)
                    x3state[1] = x3_load(1)
                    kvstate = {}
                    kvstate[0] = f4_block(0)
                    # deferred loads: first needed ~8-12us in (Q/Wr of sample 0)
                    nc.sync.dma_start(bqc_sb[:], biasq_cm.rearrange("(c p) t -> p c t", p=128))
                    nc.sync.dma_start(wq_sb[:], wq.rearrange("(c p) d -> p c d", p=128))
                    nc.sync.dma_start(wr_sb[:], wr.rearrange("(c p) d -> p c d", p=128))
                    nc.sync.dma_start(brrow_sb[:], br_row)
                    if NG4 > 1:
                        kvstate[1] = f4_block(1)
                    # W1/W2 loads deferred here so startup DMA bandwidth goes
                    # to the first groups' activations and attention weights.
                    nc.sync.dma_start(w1_sb[:], w1.rearrange("(c p) d -> p c d", p=128))
                    nc.sync.dma_start(w2_sb[:], w2.rearrange("(c i p) d -> p c i d", p=128, i=2))
                    nc.sync.dma_start(b1_sb[:], bias1_cm)
                    nc.sync.dma_start(b2row_sb[:], b2_row)
                    for g in range(NG4):
                        k_tm, v_tm, kr_cm = kvstate.pop(g)
                        # ---------- f3 blocks: SG4 samples ----------
                        for sp in range(SG4 // 2):
                            if sp == 2 and g + 2 < NG4:
                                kvstate[g + 2] = f4_block(g + 2)
                            s0 = SG4 * g + 2 * sp
                            xs, x3hs = [], []
                            for sl in range(2):
                                s = s0 + sl
                                x3 = x3state.pop(s)
                                if s + 2 < NS:
                                    x3state[s + 2] = x3_load(s + 2)
                                mv1 = p1.tile([128, 2, 2], F32, tag="mv1", bufs=3)
                                for t in range(2):
                                    bns1 = p1.tile([128, 6], F32, tag="bns1", bufs=3)
                                    nc.vector.bn_stats(bns1[:], x3[:, t, :])
                                    nc.vector.bn_aggr(mv1[:, t, :], bns1[:])
                                negm1 = p1.tile([128, 2], F32, tag="negm1", bufs=3)
                                nc.vector.tensor_scalar_mul(negm1[:], mv1[:, :, 0], -1.0)
                                sinv1 = p1.tile([128, 2], F32, tag="sinv1", bufs=3)
                                inv_std_from_var(mv1[:, :, 1], sinv1[:], 2, "s1")
                                negmn1 = p1.tile([128, 2], F32, tag="negmn1", bufs=3)
                                nc.vector.tensor_tensor(negmn1[:], negm1[:], sinv1[:], op=ALU.mult)
                                x3h = p1.tile([128, 2, DIM], BF16, tag="x3h", bufs=4)
                                for t in range(2):
                                    nc.scalar.activation(
                                        x3h[:, t, :], x3[:, t, :], ACTF.Identity,
                                        bias=negmn1[:, t: t + 1], scale=sinv1[:, t: t + 1],
                                    )
                                xs.append(x3); x3hs.append(x3h)
                            # pair channel-major LN'd x3: [512ch, 512tok]
                            x3c = p1.tile([128, 4, 512], FP8, tag="x3c", bufs=3)
                            for cc in range(4):
                                ptpf3 = ps_tp.tile([128, 512], F32R, tag="tp", name="ptpf3")
                                ptp = ptpf3.bitcast(BF16)
                                for sl in range(2):
                                    for t in range(2):
                                        nc.tensor.transpose(
                                            ptp[:, sl * 256 + t * 128: sl * 256 + (t + 1) * 128],
                                            x3hs[sl][:, t, cc * 128:(cc + 1) * 128],
                                            identb_sb[:],
                                        )
                                nc.scalar.activation(x3c[:, cc, :], ptp[:, 0:512], ACTF.Copy)
                            # Q projection channel-major (both samples) + bias + exp
                            qexp = p1.tile([128, 4, 512], BF16, tag="qexp", bufs=3)
                            qsum = ps_sm.tile([8, DIM], F32, tag="sm")
                            for cc in range(4):
                                pqc = ps_mmq.tile([128, 512], F32, tag="mmq", name="pqc")
                                nc.tensor.matmul(
                                    pqc[:], identb_sb[:], bqc_sb[:, cc, :],
                                    start=True, stop=False,
                                )
                                for j in range(2):
                                    nc.tensor.matmul(
                                        pqc[:],
                                        wq_sb[:, 2 * j: 2 * j + 2, cc * 128:(cc + 1) * 128],
                                        x3c[:, 2 * j: 2 * j + 2, :],
                                        start=False, stop=(j == 1), perf_mode=DR,
                                    )
                                nc.scalar.activation(qexp[:, cc, :], pqc[:], ACTF.Exp, scale=RS)
                                nc.tensor.matmul(
                                    qsum[:], hsel_sb[:, cc, :], qexp[:, cc, :],
                                    start=(cc == 0), stop=(cc == 3),
                                    skip_group_check=True,
                                )
                            qre = p1.tile([8, DIM], BF16, tag="qre", bufs=3)
                            with nc.allow_low_precision(reason="softmax 1/sum in bf16"):
                                nc.vector.reciprocal(qre[:], qsum[:])
                            # attention per head-pair, both samples
                            att2 = p1.tile([128, 4, 512], FP8, tag="att_cm", bufs=3)
                            for hp in range(4):
                                arb = ps_mmq.tile([128, 512], F32, tag="mmq", name="arb")
                                nc.tensor.matmul(
                                    arb[:], hexp_sb[:, hp, :], qre[:],
                                    start=True, stop=True,
                                )
                                apt = ps_mmq.tile([128, 512], F32, tag="mmq", name="apt")
                                qn2 = p1.tile([128, 512], BF16, tag="qn", bufs=4)
                                nc.vector.tensor_tensor(
                                    qn2[:], qexp[:, hp, :], arb[:], op=ALU.mult,
                                )
                                for sl in range(2):
                                    s = s0 + sl
                                    si = 2 * sp + sl
                                    tb = si // 2
                                    pb = (si % 2) * 64
                                    ck2 = ps_ctx.tile([128, 144], F32R, tag="ctxpkr")
                                    pctx = ck2.bitcast(F32)[:, 0:128]
                                    nc.tensor.matmul(
                                        pctx,
                                        k_tm[pb:pb + 64, tb, hp * 128:(hp + 1) * 128],
                                        v_tm[pb:pb + 64, tb, hp * 128:(hp + 1) * 128],
                                        start=True, stop=True,
                                    )
                                    ctxbd = ctxbd2[:, sl]
                                    for hh in range(2):
                                        nc.vector.tensor_scalar_mul(
                                            ctxbd[hh * 64:(hh + 1) * 64, hp, hh * 64:(hh + 1) * 64],
                                            pctx[hh * 64:(hh + 1) * 64, hh * 64:(hh + 1) * 64],
                                            kr_cm[hh * 64:(hh + 1) * 64, hp, si: si + 1],
                                        )
                                    nc.tensor.matmul(
                                        apt[:, sl * 256:(sl + 1) * 256],
                                        ctxbd[:, hp, :], qn2[:, sl * 256:(sl + 1) * 256],
                                        start=True, stop=True,
                                    )
                                nc.scalar.activation(att2[:, hp, :], apt[:], ACTF.Copy)
                            # Wr + residual -> f3out (+ LN3 stats via bn_stats)
                            for sl in range(2):
                                s = s0 + sl
                                for t in range(2):
                                    po = ps_mmr.tile([128, 512], F32, tag="mmr")
                                    if not zero_rows:
                                        nc.tensor.matmul(po[:], ones_sb[:], brrow_sb[:], start=True, stop=False)
                                    for j in range(2):
                                        nc.tensor.matmul(
                                            po[:],
                                            att2[:, 2 * j: 2 * j + 2, sl * 256 + t * 128: sl * 256 + (t + 1) * 128],
                                            wr_sb[:, 2 * j: 2 * j + 2, :],
                                            start=(zero_rows and j == 0), stop=(j == 1), perf_mode=DR,
                                        )
                                    f3o = p13.tile([128, DIM], BF16, tag="f3o", bufs=4)
                                    nc.vector.scalar_tensor_tensor(
                                        f3o[:], po[:], RS, xs[sl][:, t, :],
                                        op0=ALU.mult, op1=ALU.add,
                                    )
                                    bns3 = p1.tile([128, 6], F32, tag="bns3", bufs=3)
                                    nc.vector.bn_stats(bns3[:], f3o[:])
                                    nc.vector.bn_aggr(stats3[:, 2 * s + t, :], bns3[:])
                                    nc.sync.dma_start(
                                        f3o_dram[s, t * 128:(t + 1) * 128, :], f3o[:]
                                    )

                # ================= STAGE 2 =================
                with (
                    tc.tile_pool(name="s2_sb", bufs=3) as p2,
                    tc.tile_pool(name="s2_sb3", bufs=3) as p23,
                    tc.tile_pool(name="ps2_tp", bufs=2, space="PSUM") as ps2_tp,
                    tc.tile_pool(name="ps2_w1", bufs=2, space="PSUM") as ps2_w1,
                    tc.tile_pool(name="ps2_w2", bufs=4, space="PSUM") as ps2_w2,
                ):
                    # LN3 stats math for all samples at once
                    negm3 = p2.tile([128, 2 * NS], F32, tag="negm3")
                    nc.vector.tensor_scalar_mul(negm3[:], stats3[:, :, 0], -1.0)
                    lnv3 = p2.tile([128, 2 * NS], F32, tag="lnv3")
                    nc.scalar.activation(lnv3[:], stats3[:, :, 1], ACTF.Ln, bias=eps_sb[:])
                    s3 = p2.tile([128, 2 * NS], F32, tag="s3")
                    nc.scalar.activation(s3[:], lnv3[:], ACTF.Exp, scale=-0.5)

                    for g in range(NG2):
                        f3o2 = p23.tile([128, 4, DIM], BF16, tag="f3o2")
                        for c in range(4):
                            nc.sync.dma_start(
                                f3o2[:, c, :],
                                f3o_dram[2 * g + c // 2, (c % 2) * 128:(c % 2) * 128 + 128, :],
                            )
                        xoh = p2.tile([128, 4, DIM], BF16, tag="xoh")
                        for c in range(4):
                            col = 4 * g + c
                            nc.vector.tensor_scalar(
                                xoh[:, c, :], f3o2[:, c, :],
                                negm3[:, col: col + 1], s3[:, col: col + 1],
                                op0=ALU.add, op1=ALU.mult,
                            )
                        xoc = p2.tile([128, 4, DIM], FP8, tag="xoc")
                        for cc in range(4):
                            ptpf2 = ps2_tp.tile([128, 512], F32R, tag="tp2", name="ptpf2")
                            ptp = ptpf2.bitcast(BF16)
                            for c in range(4):
                                nc.tensor.transpose(
                                    ptp[:, c * 128:(c + 1) * 128],
                                    xoh[:, c, cc * 128:(cc + 1) * 128],
                                    identb_sb[:],
                                )
                            nc.vector.tensor_copy(xoc[:, cc, :], ptp[:, 0:512])
                        pf = []
                        for c in range(4):
                            pfc = ps2_w2.tile([128, 512], F32, tag="w2acc")
                            if not zero_rows:
                                nc.tensor.matmul(pfc[:], ones_sb[:], b2row_sb[:], start=True, stop=False)
                            pf.append(pfc)
                        for hcp in range(8):
                            gt2 = p23.tile([128, 2, DIM], FP8, tag="gt2")
                            for i in range(2):
                                hc = 2 * hcp + i
                                pw1 = ps2_w1.tile([128, 512], F32, tag="w1ps")
                                for j in range(2):
                                    nc.tensor.matmul(
                                        pw1[:],
                                        w1_sb[:, 2 * j: 2 * j + 2, hc * 128:(hc + 1) * 128],
                                        xoc[:, 2 * j: 2 * j + 2, :],
                                        start=(j == 0), stop=(j == 1), perf_mode=DR,
                                    )
                                nc.scalar.activation(
                                    gt2[:, i, :], pw1[:], ACTF.Gelu,
                                    bias=b1_sb[:, hc: hc + 1], scale=RS,
                                )
                            for c in range(4):
                                nc.tensor.matmul(
                                    pf[c][:],
                                    gt2[:, :, c * 128:(c + 1) * 128],
                                    w2_sb[:, hcp, :, :],
                                    start=(zero_rows and hcp == 0), stop=(hcp == 7), perf_mode=DR,
                                    skip_group_check=True,
                                )
                        for c in range(4):
                            outt = p2.tile([128, DIM], F32, tag="outt")
                            nc.vector.scalar_tensor_tensor(
                                outt[:], pf[c][:], RS, f3o2[:, c, :],
                                op0=ALU.mult, op1=ALU.add,
                            )
                            nc.sync.dma_start(
                                out[2 * g + c // 2, (c % 2) * 128:(c % 2) * 128 + 128, :],
                                outt[:],
                            )

    nc.compile()
    return nc


def _get_module(n_samples, zero_rows=False):
    key = (n_samples, zero_rows)
    if key not in _BUILD_CACHE:
        _BUILD_CACHE[key] = _build(n_samples, zero_rows=zero_rows)
    return _BUILD_CACHE[key]


def kernel(**inputs) -> np.ndarray:
    from concourse.bass_utils import run_bass_kernel_spmd

    consts = _host_prep(inputs)
    f3 = np.ascontiguousarray(np.asarray(inputs["f3"], dtype=np.float32))
    f4 = np.ascontiguousarray(np.asarray(inputs["f4"], dtype=np.float32))

    zero_rows = (
        not np.any(consts["biasv_row"]) and not np.any(consts["br_row"])
        and not np.any(consts["b2_row"])
    )
    nc = _get_module(BSH, zero_rows)
    in_maps = []
    for c in range(N_CORES):
        m = dict(consts)
        m["f3"] = np.ascontiguousarray(f3[c * BSH:(c + 1) * BSH])
        m["f4"] = np.ascontiguousarray(f4[c * BSH:(c + 1) * BSH])
        in_maps.append(m)
    res = run_bass_kernel_spmd(nc, in_maps, core_ids=list(range(N_CORES)))
    return np.concatenate([res.results[c]["out"] for c in range(N_CORES)], axis=0)
